# revision 5
# baseline (speedup 1.0000x reference)
"""Linear-attention recurrent-state kernel for Trainium2 (Bass/Tile), SPMD over 8 cores.

Math per token t=(n,h):
    q,k,v = x W^T + b;  kq = elu(q)+1;  kk = elu(k)+1
    Zi_new = Zi + kk
    Si_new = Si + kk v^T            (rank-1 update, 256x256 fp32)
    Z = 1/(kq . Zi_new + eps)
    out = (Z * kq . Si_new) Wo^T + bo

Sharding: pure data parallel over N (64 n -> 256 tokens per core).
Memory-bound: Si/Si_new stream (64+64 MiB per core) dominates.

Device mapping per core:
  phase A (batched, feature-major [feat(2x128 part), tok(256 free)]):
      projections on PE (fp32), elu+1 = exp(min(x,0))+relu(x) on DVE/ACT,
      Zi_new, Z row, kq_scaled = kq*Z, token-major copies of v via PE transpose.
  phase B loop (32 groups x 8 tokens): DMA Si group in (2x1MiB),
      per token: GPSIMD broadcast of v row -> fused DVE scalar_tensor_tensor
      Si_new = VB*kk[d] + Si (in-place), PE matvecs Si_new^T kq_scaled into
      V^T psum columns; DMA Si_new group out.
  phase C: V^T -> out^T = Wo^T-contract + bias, PE transpose to token-major, DMA out.
"""

import numpy as np
from contextlib import ExitStack

import concourse.bass as bass
import concourse.tile as tile
from concourse import bacc, mybir
from concourse import bass_utils

N, H, E = 512, 4, 256
NCORES = 8
NPC = N // NCORES          # n per core
T = NPC * H                # tokens per core = 256
EPS = 1e-6
F32 = mybir.dt.float32
AF = mybir.ActivationFunctionType
ALU = mybir.AluOpType
GROUP = 8                  # tokens per DMA group
NGROUPS = T // GROUP
HALF = 128


def _emit(ctx: ExitStack, tc: tile.TileContext, io: dict):
    nc = tc.nc
    const = ctx.enter_context(tc.tile_pool(name="const", bufs=1))
    work = ctx.enter_context(tc.tile_pool(name="work", bufs=2))
    sip = ctx.enter_context(tc.tile_pool(name="sip", bufs=3))
    vbp = ctx.enter_context(tc.tile_pool(name="vbp", bufs=4))
    psum = ctx.enter_context(tc.tile_pool(name="psum", bufs=2, space="PSUM"))
    psum_v = ctx.enter_context(tc.tile_pool(name="psum_v", bufs=1, space="PSUM"))

    def cdma(name, src, shape):
        t_ = const.tile(shape, F32, tag=name, name=name)
        nc.sync.dma_start(t_[:], src)
        return t_

    # ---- constants / weights (feature-major: [d, e] so contraction dim is partitions)
    ident = cdma("ident", io["ident"][:, :], [128, 128])
    wts = {}
    for w in ("wqt", "wkt", "wvt", "wot"):
        wts[w] = [cdma(f"{w}{h}", io[w][h * HALF:(h + 1) * HALF, :], [128, E]) for h in range(2)]
    bias = {}
    for b in ("bq", "bk", "bv", "bo"):
        bias[b] = [cdma(f"{b}{h}", io[b][h * HALF:(h + 1) * HALF, :], [128, 1]) for h in range(2)]
    xt = [cdma(f"xt{h}", io["xt"][h * HALF:(h + 1) * HALF, :], [128, T]) for h in range(2)]
    zit = [cdma(f"zit{h}", io["zit"][h * HALF:(h + 1) * HALF, :], [128, T]) for h in range(2)]
    ones = const.tile([128, 1], F32, tag="ones", name="ones")
    nc.vector.memset(ones[:], 1.0)

    # ---- phase A: projections (feature-major result [e(128x2), t(256)])
    def project(wname, bname, apply_kernel, out_tag):
        """returns 2 sbuf tiles [128, T]; if apply_kernel: elu(.)+1 else +bias only"""
        outs = []
        for eh in range(2):
            ps = psum.tile([128, T], F32, tag="proj", name="proj")
            for dh in range(2):
                nc.tensor.matmul(
                    ps[:], lhsT=wts[wname][dh][:, eh * HALF:(eh + 1) * HALF],
                    rhs=xt[dh][:], start=(dh == 0), stop=(dh == 1))
            o = const.tile([128, T], F32, tag=f"{out_tag}{eh}", name=f"{out_tag}{eh}")
            if not apply_kernel:
                nc.vector.tensor_scalar_add(o[:], ps[:], bias[bname][eh][:])
            else:
                # elu(x)+1 = exp(min(x+b,0)) + relu(x+b), exact both branches
                mn = work.tile([128, T], F32, tag="mn", name="mn")
                nc.vector.tensor_scalar(
                    out=mn[:], in0=ps[:], scalar1=bias[bname][eh][:], scalar2=0.0,
                    op0=ALU.add, op1=ALU.min)
                ex = work.tile([128, T], F32, tag="ex", name="ex")
                nc.scalar.activation(ex[:], mn[:], AF.Exp)
                rl = work.tile([128, T], F32, tag="rl", name="rl")
                nc.scalar.activation(rl[:], ps[:], AF.Relu, bias=bias[bname][eh][:])
                nc.vector.tensor_add(o[:], ex[:], rl[:])
            outs.append(o)
        return outs

    kq = project("wqt", "bq", True, "kq")
    kk = project("wkt", "bk", True, "kk")
    vt = project("wvt", "bv", False, "vt")     # [m, t] feature-major v

    def transpose_256(src, dst_tag, copy_engine):
        """src: 2 tiles [128, T] = [r(2x128), c(256)] -> dst 2 tiles [c(2x128), r(256)]"""
        dst = [const.tile([128, 256], F32, tag=f"{dst_tag}{h}", name=f"{dst_tag}{h}") for h in range(2)]
        for ch in range(2):
            for rh in range(2):
                ps = psum.tile([128, 128], F32, tag="tr", name="tr")
                nc.tensor.transpose(ps[:], src[rh][:, ch * HALF:(ch + 1) * HALF], ident[:])
                copy_engine(dst[ch][:, rh * HALF:(rh + 1) * HALF], ps[:])
        return dst

    sc_copy = lambda o, i: nc.scalar.copy(o, i)

    # Zi_new = Zi + kk (feature-major), store token-major
    zin = []
    for eh in range(2):
        z = work.tile([128, T], F32, tag=f"zin{eh}", name=f"zin{eh}")
        nc.vector.tensor_add(z[:], zit[eh][:], kk[eh][:])
        zin.append(z)
    zi_tok = transpose_256(zin, "zitok", sc_copy)
    for th in range(2):
        nc.sync.dma_start(io["zi_out"][th * HALF:(th + 1) * HALF, :], zi_tok[th][:])

    # denom row: sum_e kq*Zi_new  -> Z = 1/(denom+eps)
    den_ps = psum_v.tile([1, T], F32, tag="den", name="den")
    for eh in range(2):
        prod = work.tile([128, T], F32, tag="prod", name="prod")
        nc.vector.tensor_mul(prod[:], kq[eh][:], zin[eh][:])
        nc.tensor.matmul(den_ps[:], lhsT=ones[:], rhs=prod[:],
                         start=(eh == 0), stop=(eh == 1))
    den = const.tile([1, T], F32, tag="den_s", name="den_s")
    nc.vector.tensor_scalar_add(den[:], den_ps[:], float(EPS))
    zrow = const.tile([1, T], F32, tag="zrow", name="zrow")
    nc.vector.reciprocal(zrow[:], den[:])
    zb = const.tile([128, T], F32, tag="zb", name="zb")
    nc.gpsimd.partition_broadcast(zb[:], zrow[:])
    kqs = []
    for eh in range(2):
        s = const.tile([128, T], F32, tag=f"kqs{eh}", name=f"kqs{eh}")
        nc.vector.tensor_mul(s[:], kq[eh][:], zb[:])
        kqs.append(s)

    # token-major v rows for the broadcast source
    vtok = transpose_256(vt, "vtok", sc_copy)

    # ---- phase B: stream Si, rank-1 update, readout V^T columns
    si_d = io["si_in"].rearrange("t d m -> d t m")
    so_d = io["si_out"].rearrange("t d m -> d t m")
    vt_ps = [psum_v.tile([128, T], F32, tag=f"vtps{mh}", name=f"vtps{mh}") for mh in range(2)]

    for g in range(NGROUPS):
        t0 = g * GROUP
        si = []
        for dh in range(2):
            s = sip.tile([128, GROUP * 256], F32, tag=f"si{dh}", name=f"si{dh}")
            sv = s[:].rearrange("p (g m) -> p g m", m=256)
            nc.sync.dma_start(sv, si_d[dh * HALF:(dh + 1) * HALF, t0:t0 + GROUP, :])
            si.append(s)
        # engines need APs at partition 0/32/64/96: gather this group's v rows
        # (scattered across partitions of vtok) onto partition 0 via DMA
        vrow = vbp.tile([1, GROUP * 256], F32, tag="vrow", name="vrow")
        r0 = t0 % HALF
        nc.sync.dma_start(
            vrow[:].rearrange("p (g m) -> p g m", m=256),
            vtok[t0 // HALF][r0:r0 + GROUP, :])
        for i in range(GROUP):
            t = t0 + i
            vb = vbp.tile([128, 256], F32, tag="vb", name="vb")
            nc.gpsimd.partition_broadcast(vb[:], vrow[0:1, i * 256:(i + 1) * 256])
            for dh in range(2):
                blk = si[dh][:, i * 256:(i + 1) * 256]
                nc.vector.scalar_tensor_tensor(
                    out=blk, in0=vb[:], scalar=kk[dh][:, t:t + 1], in1=blk,
                    op0=ALU.mult, op1=ALU.add)
            for mh in range(2):
                for dh in range(2):
                    nc.tensor.matmul(
                        vt_ps[mh][:, t:t + 1],
                        lhsT=si[dh][:, i * 256 + mh * HALF: i * 256 + (mh + 1) * HALF],
                        rhs=kqs[dh][:, t:t + 1],
                        start=(dh == 0), stop=(dh == 1))
        for dh in range(2):
            sv = si[dh][:].rearrange("p (g m) -> p g m", m=256)
            nc.sync.dma_start(so_d[dh * HALF:(dh + 1) * HALF, t0:t0 + GROUP, :], sv)

    # ---- phase C: out^T[e,t] = sum_m WoT[m,e] V^T[m,t] + bo; -> token-major; DMA
    vts = []
    for mh in range(2):
        v = work.tile([128, T], F32, tag=f"vts{mh}", name=f"vts{mh}")
        nc.vector.tensor_copy(v[:], vt_ps[mh][:])
        vts.append(v)
    outT = []
    for eh in range(2):
        ps = psum.tile([128, T], F32, tag="proj", name="proj")
        for mh in range(2):
            nc.tensor.matmul(ps[:], lhsT=wts["wot"][mh][:, eh * HALF:(eh + 1) * HALF],
                             rhs=vts[mh][:], start=(mh == 0), stop=(mh == 1))
        o = work.tile([128, T], F32, tag=f"outT{eh}", name=f"outT{eh}")
        nc.vector.tensor_scalar_add(o[:], ps[:], bias["bo"][eh][:])
        outT.append(o)
    o_tok = transpose_256(outT, "otok", sc_copy)
    for th in range(2):
        nc.sync.dma_start(io["o_out"][th * HALF:(th + 1) * HALF, :], o_tok[th][:])


def build_program():
    nc = bacc.Bacc("TRN2", target_bir_lowering=False, debug=False,
                   enable_asserts=False, num_devices=NCORES)
    io = {}
    io["si_in"] = nc.dram_tensor("si_in", (T, E, E), F32, kind="ExternalInput").ap()
    io["xt"] = nc.dram_tensor("xt", (E, T), F32, kind="ExternalInput").ap()
    io["zit"] = nc.dram_tensor("zit", (E, T), F32, kind="ExternalInput").ap()
    for w in ("wqt", "wkt", "wvt", "wot"):
        io[w] = nc.dram_tensor(w, (E, E), F32, kind="ExternalInput").ap()
    for b in ("bq", "bk", "bv", "bo"):
        io[b] = nc.dram_tensor(b, (E, 1), F32, kind="ExternalInput").ap()
    io["ident"] = nc.dram_tensor("ident", (128, 128), F32, kind="ExternalInput").ap()
    io["si_out"] = nc.dram_tensor("si_out", (T, E, E), F32, kind="ExternalOutput").ap()
    io["zi_out"] = nc.dram_tensor("zi_out", (T, E), F32, kind="ExternalOutput").ap()
    io["o_out"] = nc.dram_tensor("o_out", (T, E), F32, kind="ExternalOutput").ap()

    with tile.TileContext(nc) as tc:
        with ExitStack() as ctx:
            _emit(ctx, tc, io)
    nc.compile()
    return nc


def make_in_maps(x, Si, Zi, Wq, bq, Wk, bk, Wv, bv, Wo, bo):
    f = np.float32
    shared = {
        "wqt": np.ascontiguousarray(np.asarray(Wq, f).T),
        "wkt": np.ascontiguousarray(np.asarray(Wk, f).T),
        "wvt": np.ascontiguousarray(np.asarray(Wv, f).T),
        "wot": np.ascontiguousarray(np.asarray(Wo, f).T),
        "bq": np.asarray(bq, f).reshape(E, 1).copy(),
        "bk": np.asarray(bk, f).reshape(E, 1).copy(),
        "bv": np.asarray(bv, f).reshape(E, 1).copy(),
        "bo": np.asarray(bo, f).reshape(E, 1).copy(),
        "ident": np.eye(128, dtype=f),
    }
    x = np.asarray(x, f)
    Si = np.asarray(Si, f)
    Zi = np.asarray(Zi, f)
    maps = []
    for c in range(NCORES):
        sl = slice(c * NPC, (c + 1) * NPC)
        maps.append(dict(
            shared,
            si_in=np.ascontiguousarray(Si[sl].reshape(T, E, E)),
            xt=np.ascontiguousarray(x[sl].reshape(T, E).T),
            zit=np.ascontiguousarray(Zi[sl].reshape(T, E).T),
        ))
    return maps


_PROGRAM = None


def _program():
    global _PROGRAM
    if _PROGRAM is None:
        _PROGRAM = build_program()
    return _PROGRAM


def run(in_maps, trace=False, **kw):
    nc = _program()
    return bass_utils.run_bass_kernel_spmd(nc, in_maps, core_ids=list(range(NCORES)),
                                           trace=trace, **kw)


def kernel(x, Si, Zi, Wq, bq, Wk, bk, Wv, bv, Wo, bo):
    in_maps = make_in_maps(x, Si, Zi, Wq, bq, Wk, bk, Wv, bv, Wo, bo)
    res = run(in_maps)
    outs, si_news, zi_news = [], [], []
    for c in range(NCORES):
        r = res.results[c]
        outs.append(r["o_out"].reshape(NPC, H, E))
        si_news.append(r["si_out"].reshape(NPC, H, E, E))
        zi_news.append(r["zi_out"].reshape(NPC, H, E))
    out = np.concatenate(outs, axis=0)
    si_new = np.concatenate(si_news, axis=0)
    zi_new = np.concatenate(zi_news, axis=0)
    return out, si_new, zi_new
